# revision 7
# baseline (speedup 1.0000x reference)
"""Trainium2 Bass kernel for the BINN convnet problem.

Computation (per row b of inp, all column indices mod D=128):
    x[b, j]  = (c1[j] * a[b, j+1] - c2[j] * a[b, j-2]) * a[b, j-1]
    out      = x + a @ W_lin.T + b_lin
with c1[j] = w[j,0]*w[j,2], c2[j] = w[j,1]*w[j,2], except j==1 where the
outer factor is w[1,0] instead of w[1,2].

Strategy: pure data parallel across 8 NeuronCores (batch split).  On each
core, per 512-row compute subtile (1024-row DMA tiles, (p q) layout so each
partition line is one contiguous 4 KB DRAM chunk):

  1. PE-transposes A -> A^T per 128-row block (plain fp32 transpose mode);
     ScalarE evacuates PSUM->SBUF, rounding to float32r.
  2. The stencil's linear part g[b,j] = c1[j] a[b,j+1] - c2[j] a[b,j-2] is
     a constant banded matrix G.  One float32r matmul per block with
     lhsT = A^T-block (stationary) and rhs = [G^T | W_lin^T] (moving,
     N=256 -> full PE rate) produces g and mm = a @ W_lin.T both in
     NATURAL layout in PSUM.  No transpose-back is needed.
  3. DVE: x = a[:, j-1] * g with the j-1 roll expressed as shifted
     free-dim access patterns on the natural A tile (main + 1-col wrap),
     then out = x + mm written straight to SBUF.
  4. GpSimd adds the column bias b_lin (broadcast constant) in SBUF.
  5. Store the natural tile.
"""

import os
import sys

import numpy as np

if os.path.isdir("/opt/trn_rl_repo") and "/opt/trn_rl_repo" not in sys.path:
    sys.path.insert(0, "/opt/trn_rl_repo")

import concourse.mybir as mybir
import concourse.tile as tile
from concourse import bacc
from concourse.bass_utils import run_bass_kernel_spmd

D = 128          # feature dim
N_CORES = 8
SUB = 512        # rows per compute subtile
DMA_ROWS = 2048  # rows per DMA tile
F32 = mybir.dt.float32
F32R = mybir.dt.float32r
F16 = mybir.dt.float16
BIAS_ON_POOL = True


def build_program(nrows: int):
    """Build the single-core Bass program (SPMD across cores)."""
    assert nrows % DMA_ROWS == 0
    ndma = nrows // DMA_ROWS
    nsub = DMA_ROWS // SUB  # compute subtiles per DMA tile (2)
    QB = SUB // D           # 128-row blocks per compute subtile (4)

    nc = bacc.Bacc("TRN2", debug=False, target_bir_lowering=False)

    inp = nc.declare_dram_parameter("inp", [nrows, D], F32, isOutput=False)
    gw = nc.declare_dram_parameter("gw", [D, 2 * D], F32, isOutput=False)
    bbc = nc.declare_dram_parameter("bbc", [D, DMA_ROWS], F32, isOutput=False)
    bmask = nc.declare_dram_parameter("bmask", [1, SUB], F32, isOutput=False)
    ones = nc.declare_dram_parameter("ones", [1, D], F32, isOutput=False)
    ident = nc.declare_dram_parameter("ident", [D, D], F32, isOutput=False)
    out = nc.declare_dram_parameter("out", [nrows, D], F16, isOutput=True)

    with tile.TileContext(nc) as tc:
        with (
            tc.tile_pool(name="const", bufs=1) as const_pool,
            tc.tile_pool(name="a_sb", bufs=6) as a_pool,
            tc.tile_pool(name="at_sb", bufs=4) as at_pool,
            tc.tile_pool(name="xt_sb", bufs=4) as xt_pool,
            tc.tile_pool(name="o_sb", bufs=6) as o_pool,
            tc.tile_pool(name="at_ps", bufs=2, space="PSUM") as atps_pool,
            tc.tile_pool(name="gw_ps", bufs=3, space="PSUM") as gwps_pool,
        ):
            # --- constants, loaded once ---
            gw_sb = const_pool.tile([D, 2 * D], F32)
            bbc_sb = const_pool.tile([D, DMA_ROWS], F32)
            bmask_sb = const_pool.tile([1, SUB], F32)
            ones_sb = const_pool.tile([1, D], F32)
            id_sb = const_pool.tile([D, D], F32)
            nc.sync.dma_start(out=gw_sb[:], in_=gw[:, :])
            nc.sync.dma_start(out=bbc_sb[:], in_=bbc[:, :])
            nc.sync.dma_start(out=bmask_sb[:], in_=bmask[:, :])
            nc.sync.dma_start(out=ones_sb[:], in_=ones[:, :])
            nc.sync.dma_start(out=id_sb[:], in_=ident[:, :])

            # fp32r matmul operands must be produced by an fp32r-rounding
            # instruction (walrus checkMatmultFP32r) — round the constants once.
            gw_rt = const_pool.tile([D, 2 * D], F32R)
            bmask_rt = const_pool.tile([1, SUB], F32R)
            ones_rt = const_pool.tile([1, D], F32R)
            id_rt = const_pool.tile([D, D], F32R)
            nc.vector.tensor_copy(out=gw_rt[:], in_=gw_sb[:])
            nc.vector.tensor_copy(out=bmask_rt[:], in_=bmask_sb[:])
            nc.vector.tensor_copy(out=ones_rt[:], in_=ones_sb[:])
            nc.vector.tensor_copy(out=id_rt[:], in_=id_sb[:])

            # Software pipeline by one subtile: PE's stream per step is
            # [trA(k), GW(k-1)] so PE transposes subtile k while ScalarE
            # evacuates A^T of k-1 — no PE stall on the evac round-trip.
            nsubs = ndma * nsub
            st = {}  # k -> (td, f0, a_sb, o_sb, at_ps, at_sb)

            def emit_front(k):
                td, ts = divmod(k, nsub)
                if ts == 0:
                    r0 = td * DMA_ROWS
                    # (p q) layout: partition p holds DMA_ROWS/128 consecutive
                    # DRAM rows -> one contiguous DRAM chunk per partition.
                    a_sb = a_pool.tile([D, DMA_ROWS], F32, tag="a")
                    src = inp[r0 : r0 + DMA_ROWS, :].rearrange(
                        "(p q) d -> p q d", p=D
                    )
                    nc.sync.dma_start(
                        out=a_sb[:].rearrange("p (q d) -> p q d", d=D), in_=src
                    )
                    o_sb = o_pool.tile([D, DMA_ROWS], F16, tag="o")
                else:
                    _, _, a_sb, o_sb, _, _ = st[k - 1]
                f0 = ts * SUB

                # --- PE transpose A -> A^T (per 128 block, plain fp32) ---
                at_ps = atps_pool.tile([D, SUB], F32, tag="atps")
                for q in range(QB):
                    nc.tensor.matmul(
                        out=at_ps[:, q * D : (q + 1) * D],
                        lhsT=a_sb[:, f0 + q * D : f0 + (q + 1) * D],
                        rhs=id_sb[:],
                        is_transpose=True,
                        start=True,
                        stop=True,
                    )
                st[k] = (td, f0, a_sb, o_sb, at_ps, None)

            def emit_evac(k):
                td, f0, a_sb, o_sb, at_ps, _ = st[k]
                # evacuate A^T to SBUF (ScalarE), rounding to fp32r
                at_sb = at_pool.tile([D, SUB], F32R, tag="at")
                nc.scalar.copy(out=at_sb[:], in_=at_ps[:])
                st[k] = (td, f0, a_sb, o_sb, at_sb, None)

            def emit_gw(k):
                td, f0, a_sb, o_sb, at_sb, _ = st[k]
                # --- [g | mm] per block, natural layout, in PSUM ---
                # gw_ps free layout: [g0|m0|g1|m1|g2|m2|g3|m3], 2 banks
                gw_ps = gwps_pool.tile([D, 4 * 2 * D], F32, tag="gwps")
                for q in range(QB):
                    nc.tensor.matmul(
                        out=gw_ps[:, q * 2 * D : (q + 1) * 2 * D],
                        lhsT=at_sb[:, q * D : (q + 1) * D],
                        rhs=gw_rt[:],
                        start=True,
                        stop=BIAS_ON_POOL,
                    )
                if not BIAS_ON_POOL:
                    # accumulate b_lin onto the mm halves (masked rhs)
                    for h in range(2):
                        nc.tensor.matmul(
                            out=gw_ps[:, h * SUB : (h + 1) * SUB],
                            lhsT=ones_rt[:],
                            rhs=bmask_rt[:],
                            start=False,
                            stop=True,
                        )
                st[k] = (td, f0, a_sb, o_sb, at_sb, gw_ps)

            def emit_mul(k):
                td, f0, a_sb, o_sb, _, gw_ps = st[k]
                gw3 = gw_ps[:].rearrange("p (q c) -> p q c", c=2 * D)
                a3 = a_sb[:, f0 : f0 + SUB].rearrange("p (q d) -> p q d", d=D)

                # --- x = a[:, j-1] * g (DVE; shifted free-dim APs) ---
                xt_sb = xt_pool.tile([D, SUB], F32, tag="xt")
                x3 = xt_sb[:].rearrange("p (q d) -> p q d", d=D)
                nc.vector.tensor_mul(
                    out=x3[:, :, 1:D], in0=a3[:, :, 0 : D - 1], in1=gw3[:, :, 1:D]
                )
                nc.vector.tensor_mul(
                    out=x3[:, :, 0:1], in0=a3[:, :, D - 1 : D], in1=gw3[:, :, 0:1]
                )

                # --- x += b_lin broadcast (GpSimd, on the fp32 SBUF tile) ---
                nc.gpsimd.tensor_tensor(
                    out=xt_sb[:],
                    in0=xt_sb[:],
                    in1=bbc_sb[:, 0:SUB],
                    op=mybir.AluOpType.add,
                )
                st[k] = (td, f0, a_sb, o_sb, xt_sb, gw_ps)

            def emit_add(k):
                td, f0, a_sb, o_sb, xt_sb, gw_ps = st[k]
                gw3 = gw_ps[:].rearrange("p (q c) -> p q c", c=2 * D)
                o3 = o_sb[:, f0 : f0 + SUB].rearrange("p (q d) -> p q d", d=D)

                # --- out = (x + bias) + mm (DVE, fp16 straight to SBUF) ---
                nc.vector.tensor_add(
                    out=o3[:, :, :], in0=xt_sb[:], in1=gw3[:, :, D : 2 * D]
                )

            def emit_store(k):
                td, _, _, o_sb, _, _ = st[k]
                if k % nsub == nsub - 1:
                    # --- store (Scalar HWDGE ring; loads use the SP ring).
                    # Deferred one extra stage so the store's semaphore wait
                    # (on the GpSimd bias) never stalls ACT's queue ahead of
                    # the next evacuations. ---
                    r0 = td * DMA_ROWS
                    dst = out[r0 : r0 + DMA_ROWS, :].rearrange(
                        "(p q) d -> p q d", p=D
                    )
                    nc.scalar.dma_start(
                        out=dst, in_=o_sb[:].rearrange("p (q d) -> p q d", d=D)
                    )

            # 6-stage pipeline:
            # [trA(k)] [evac(k-1)] [GW(k-2)] [mul+bias(k-3)] [add(k-4)] [store(k-5)]
            for step in range(nsubs + 5):
                if step < nsubs:
                    emit_front(step)
                if step >= 1 and step - 1 < nsubs:
                    emit_evac(step - 1)
                if step >= 2 and step - 2 < nsubs:
                    emit_gw(step - 2)
                if step >= 3 and step - 3 < nsubs:
                    emit_mul(step - 3)
                if step >= 4 and step - 4 < nsubs:
                    emit_add(step - 4)
                if step >= 5 and step - 5 < nsubs:
                    emit_store(step - 5)

    nc.compile()
    return nc


def make_consts(w: np.ndarray, W_lin: np.ndarray, b_lin: np.ndarray):
    """Host-side constant preparation (all tiny)."""
    w = np.asarray(w, np.float64)
    c1 = w[:, 0] * w[:, 2]
    c2 = w[:, 1] * w[:, 2]
    # column 1 uses w[1,0] as the outer factor (faithful to source)
    c1[1] = w[1, 0] * w[1, 0]
    c2[1] = w[1, 1] * w[1, 0]

    j = np.arange(D)
    G = np.zeros((D, D), np.float64)
    G[j, (j + 1) % D] += c1
    G[j, (j - 2) % D] -= c2

    gwm = np.zeros((D, 2 * D), np.float32)
    gwm[:, :D] = G.T           # gw[d, j] = G[j, d]
    gwm[:, D:] = np.asarray(W_lin, np.float64).T  # gw[d, D+j] = W_lin[j, d]

    b32 = np.asarray(b_lin, np.float32)
    bbc = np.ascontiguousarray(np.tile(b32, (D, DMA_ROWS // D)))  # [128, 1024]
    bmask = np.zeros((1, SUB), np.float32)
    bmask[0, D : 2 * D] = b32
    bmask[0, 3 * D : 4 * D] = b32
    ones = np.ones((1, D), np.float32)
    ident = np.eye(D, dtype=np.float32)
    return {"gw": gwm, "bbc": bbc, "bmask": bmask, "ones": ones, "ident": ident}


_PROGRAM_CACHE: dict[int, object] = {}
TRACE = False      # test-only: capture NTFF profile on the next kernel() call
TRACE_DIR = None   # test-only: where to keep NTFF/perfetto artifacts
LAST_RESULT = None  # test-only: BassKernelResults of the last run


def _get_program(nrows: int):
    if nrows not in _PROGRAM_CACHE:
        _PROGRAM_CACHE[nrows] = build_program(nrows)
    return _PROGRAM_CACHE[nrows]


def kernel(**inputs) -> np.ndarray:
    inp = np.ascontiguousarray(np.asarray(inputs["inp"], np.float32))
    w = np.asarray(inputs["w"], np.float32)
    W_lin = np.asarray(inputs["W_lin"], np.float32)
    b_lin = np.asarray(inputs["b_lin"], np.float32)

    B = inp.shape[0]
    assert inp.shape[1] == D and B % N_CORES == 0
    nrows = B // N_CORES

    consts = make_consts(w, W_lin, b_lin)
    shards = inp.reshape(N_CORES, nrows, D)

    nc = _get_program(nrows)
    in_maps = [{"inp": shards[i], **consts} for i in range(N_CORES)]
    res = run_bass_kernel_spmd(
        nc, in_maps, list(range(N_CORES)), trace=TRACE, tmpdir=TRACE_DIR
    )
    global LAST_RESULT
    LAST_RESULT = res
    out = np.concatenate(
        [np.asarray(res.results[i]["out"]) for i in range(N_CORES)], axis=0
    )
    return out.astype(np.float32)


if __name__ == "__main__":
    # quick smoke test on random data vs numpy
    rng = np.random.default_rng(0)
    B = N_CORES * DMA_ROWS * 2
    inp = rng.standard_normal((B, D)).astype(np.float32)
    w = rng.random((D, 3)).astype(np.float32)
    W_lin = (rng.standard_normal((D, D)) / np.sqrt(D)).astype(np.float32)
    b_lin = (rng.standard_normal(D) * 0.01).astype(np.float32)
    dt = np.ones(1, np.float32)

    actual = kernel(inp=inp, dt=dt, w=w, W_lin=W_lin, b_lin=b_lin)

    a = inp.astype(np.float64)
    c1 = (w[:, 0] * w[:, 2]).astype(np.float64)
    c2 = (w[:, 1] * w[:, 2]).astype(np.float64)
    c1[1] = w[1, 0] * w[1, 0]
    c2[1] = w[1, 1] * w[1, 0]
    ap1 = np.roll(a, -1, 1)
    am2 = np.roll(a, 2, 1)
    am1 = np.roll(a, 1, 1)
    x = (c1 * ap1 - c2 * am2) * am1
    expected = x + a @ W_lin.astype(np.float64).T + b_lin
    err = np.abs(actual - expected).max() / np.abs(expected).max()
    print("scale-relative absmax err:", err)



# revision 8
# speedup vs baseline: 1.4247x; 1.4247x over previous
"""Trainium2 Bass kernel for the BINN convnet problem — transposed pipeline (v6).

Computation (per row b of inp, all column indices mod D=128):
    x[b, j]  = (c1[j] * a[b, j+1] - c2[j] * a[b, j-2]) * a[b, j-1]
    out      = x + a @ W_lin.T + b_lin
with c1[j] = w[j,0]*w[j,2], c2[j] = w[j,1]*w[j,2], except j==1 where the
outer factor is w[1,0] instead of w[1,2].

v6 strategy: compute in TRANSPOSED (feature-on-partition) space, in a
rotated output basis o'[p, b] = out[b, (p+1) mod D]:

  x'[p, b] = g'[p, b] * aT[p, b]        (partition-aligned! no shifts)
  g'       = RotG @ aT   (RotG[p,:] = G[p+1,:], constant STATIONARY)
  mm'      = RotW @ aT   (RotW[p,:] = W_lin[p+1,:])
  o'       = x' + mm' + bias'[p]        (bias is PER-PARTITION here)

Per 512-row subtile:
  1. GpSimd converts the natural fp32 tile to fp16 (SBUF->SBUF).
  2. PE transposes the fp16 tile per 128-block (1 cycle/row at fp16).
  3. ScalarE evacuates aT (PSUM->SBUF, fp16).
  4. PE: P = RotG16 @ aT (start=True, stop=False), moving N=512 fp16.
  5. DVE: P *= aT in place (PSUM read-modify-write).
  6. PE: P += RotW16 @ aT (start=False, stop=True) -- PSUM accumulation
     performs the x + mm add for free.
  7. ScalarE/DVE split: o' = P + bias' (per-partition bias), fp16 to SBUF.
  8. Store o' to a transposed DRAM output [D, nrows]; the host undoes the
     (transpose, 16-row interleave, +1 column rotation) while upcasting.

Memory: reads 33.5 MB fp32, writes 16.8 MB fp16 per core -> ~145 us DMA
roofline at the measured ~355 GB/s per-core DMA rate.
"""

import os
import sys

import numpy as np

if os.path.isdir("/opt/trn_rl_repo") and "/opt/trn_rl_repo" not in sys.path:
    sys.path.insert(0, "/opt/trn_rl_repo")

import concourse.mybir as mybir
import concourse.tile as tile
from concourse import bacc
from concourse.bass_utils import run_bass_kernel_spmd

D = 128          # feature dim
N_CORES = 8
SUB = 512        # rows per compute subtile
DMA_ROWS = 2048  # rows per DMA tile
QB = SUB // D    # 128-row blocks per subtile (4)
F32 = mybir.dt.float32
F16 = mybir.dt.float16
ACT_COLS = 240   # columns of the bias-copy handled by ScalarE (rest on DVE)


def build_program(nrows: int):
    assert nrows % DMA_ROWS == 0
    ndma = nrows // DMA_ROWS
    nsub = DMA_ROWS // SUB
    nsubs = ndma * nsub

    nc = bacc.Bacc("TRN2", debug=False, target_bir_lowering=False)

    inp = nc.declare_dram_parameter("inp", [nrows, D], F32, isOutput=False)
    gT = nc.declare_dram_parameter("gT", [D, D], F16, isOutput=False)
    wT = nc.declare_dram_parameter("wT", [D, D], F16, isOutput=False)
    bias = nc.declare_dram_parameter("bias", [D, 1], F32, isOutput=False)
    ident = nc.declare_dram_parameter("ident", [D, D], F16, isOutput=False)
    outT = nc.declare_dram_parameter("outT", [D, nrows], F16, isOutput=True)

    with tile.TileContext(nc) as tc:
        with (
            tc.tile_pool(name="const", bufs=1) as const_pool,
            tc.tile_pool(name="a_sb", bufs=3) as a_pool,
            tc.tile_pool(name="ab_sb", bufs=4) as ab_pool,
            tc.tile_pool(name="at_sb", bufs=5) as at_pool,
            tc.tile_pool(name="o_sb", bufs=3) as o_pool,
            tc.tile_pool(name="at_ps", bufs=2, space="PSUM") as atps_pool,
            tc.tile_pool(name="p_ps", bufs=4, space="PSUM") as p_pool,
        ):
            gT_sb = const_pool.tile([D, D], F16)
            wT_sb = const_pool.tile([D, D], F16)
            bias_sb = const_pool.tile([D, 1], F32)
            id_sb = const_pool.tile([D, D], F16)
            nc.sync.dma_start(out=gT_sb[:], in_=gT[:, :])
            nc.sync.dma_start(out=wT_sb[:], in_=wT[:, :])
            nc.sync.dma_start(out=bias_sb[:], in_=bias[:, :])
            nc.sync.dma_start(out=id_sb[:], in_=ident[:, :])

            st = {}  # k -> dict of live tiles

            def emit_conv(k):
                td, ts = divmod(k, nsub)
                if ts == 0:
                    r0 = td * DMA_ROWS
                    a_sb = a_pool.tile([D, DMA_ROWS], F32, tag="a")
                    src = inp[r0 : r0 + DMA_ROWS, :].rearrange(
                        "(p q) d -> p q d", p=D
                    )
                    nc.sync.dma_start(
                        out=a_sb[:].rearrange("p (q d) -> p q d", d=D), in_=src
                    )
                    o_sb = o_pool.tile([D, DMA_ROWS], F16, tag="o")
                else:
                    prev = st[k - 1]
                    a_sb, o_sb = prev["a"], prev["o"]
                f0 = ts * SUB
                ab = ab_pool.tile([D, SUB], F16, tag="ab")
                nc.gpsimd.tensor_copy(out=ab[:], in_=a_sb[:, f0 : f0 + SUB])
                st[k] = {"td": td, "f0": f0, "a": a_sb, "o": o_sb, "ab": ab}

            def emit_trans(k):
                s = st[k]
                at_ps = atps_pool.tile([D, SUB], F16, tag="atps")
                ab = s["ab"]
                for q in range(QB):
                    nc.tensor.matmul(
                        out=at_ps[:, q * D : (q + 1) * D],
                        lhsT=ab[:, q * D : (q + 1) * D],
                        rhs=id_sb[:],
                        is_transpose=True,
                        start=True,
                        stop=True,
                    )
                s["atps"] = at_ps

            def emit_evac(k):
                s = st[k]
                at = at_pool.tile([D, SUB], F16, tag="at")
                nc.scalar.copy(out=at[:], in_=s["atps"][:])
                s["at"] = at

            def emit_gmm(k):
                s = st[k]
                P = p_pool.tile([D, SUB], F32, tag="p")
                nc.tensor.matmul(
                    out=P[:], lhsT=gT_sb[:], rhs=s["at"][:], start=True, stop=False
                )
                s["P"] = P

            def emit_mul(k):
                s = st[k]
                nc.vector.tensor_mul(out=s["P"][:], in0=s["P"][:], in1=s["at"][:])

            def emit_wmm(k):
                s = st[k]
                nc.tensor.matmul(
                    out=s["P"][:], lhsT=wT_sb[:], rhs=s["at"][:], start=False, stop=True
                )

            def emit_bcopy(k):
                s = st[k]
                P, o_sb, f0 = s["P"], s["o"], s["f0"]
                nc.scalar.add(
                    out=o_sb[:, f0 : f0 + ACT_COLS],
                    in_=P[:, 0:ACT_COLS],
                    add=bias_sb[:, 0:1],
                )
                nc.vector.tensor_scalar_add(
                    out=o_sb[:, f0 + ACT_COLS : f0 + SUB],
                    in0=P[:, ACT_COLS:SUB],
                    scalar1=bias_sb[:, 0:1],
                )

            def emit_store(k):
                td, ts = divmod(k, nsub)
                if ts == nsub - 1:
                    c0 = td * DMA_ROWS
                    nc.scalar.dma_start(
                        out=outT[:, c0 : c0 + DMA_ROWS], in_=st[k]["o"][:]
                    )

            # 8-stage pipeline; per engine, oldest work is emitted first.
            for step in range(nsubs + 7):
                if step >= 7 and step - 7 < nsubs:
                    emit_store(step - 7)
                if step >= 6 and step - 6 < nsubs:
                    emit_bcopy(step - 6)
                if step >= 5 and step - 5 < nsubs:
                    emit_wmm(step - 5)
                if step >= 4 and step - 4 < nsubs:
                    emit_mul(step - 4)
                if step >= 3 and step - 3 < nsubs:
                    emit_gmm(step - 3)
                if step >= 2 and step - 2 < nsubs:
                    emit_evac(step - 2)
                if step >= 1 and step - 1 < nsubs:
                    emit_trans(step - 1)
                if step < nsubs:
                    emit_conv(step)

    nc.compile()
    return nc


def make_consts(w: np.ndarray, W_lin: np.ndarray, b_lin: np.ndarray):
    w = np.asarray(w, np.float64)
    c1 = w[:, 0] * w[:, 2]
    c2 = w[:, 1] * w[:, 2]
    c1[1] = w[1, 0] * w[1, 0]
    c2[1] = w[1, 1] * w[1, 0]

    j = np.arange(D)
    G = np.zeros((D, D), np.float64)
    G[j, (j + 1) % D] += c1
    G[j, (j - 2) % D] -= c2

    rot = (j + 1) % D  # output partition p holds natural column p+1
    RotG = G[rot, :]
    RotW = np.asarray(W_lin, np.float64)[rot, :]
    gT = np.ascontiguousarray(RotG.T).astype(np.float16)
    wT = np.ascontiguousarray(RotW.T).astype(np.float16)
    bias = np.asarray(b_lin, np.float32)[rot].reshape(D, 1)
    ident = np.eye(D, dtype=np.float16)
    return {"gT": gT, "wT": wT, "bias": bias, "ident": ident}


_PROGRAM_CACHE: dict[int, object] = {}
TRACE = False
TRACE_DIR = None
LAST_RESULT = None


def _get_program(nrows: int):
    if nrows not in _PROGRAM_CACHE:
        _PROGRAM_CACHE[nrows] = build_program(nrows)
    return _PROGRAM_CACHE[nrows]


def _unscramble(shard_t: np.ndarray, nrows: int) -> np.ndarray:
    """[D, nrows] fp16 device output -> [nrows, D] natural-layout fp16.

    Device column c = td*2048 + s16*128 + n holds row td*2048 + n*16 + s16;
    device partition p holds natural output column (p+1) mod D.
    """
    T = nrows // DMA_ROWS
    V = shard_t.reshape(D, T, DMA_ROWS // D, D)  # [p, td, s16, n]
    U = V.transpose(1, 3, 2, 0).reshape(nrows, D)  # [row, p]
    return np.roll(U, 1, axis=1)


def kernel(**inputs) -> np.ndarray:
    inp = np.ascontiguousarray(np.asarray(inputs["inp"], np.float32))
    w = np.asarray(inputs["w"], np.float32)
    W_lin = np.asarray(inputs["W_lin"], np.float32)
    b_lin = np.asarray(inputs["b_lin"], np.float32)

    B = inp.shape[0]
    assert inp.shape[1] == D and B % N_CORES == 0
    nrows = B // N_CORES

    consts = make_consts(w, W_lin, b_lin)
    shards = inp.reshape(N_CORES, nrows, D)

    nc = _get_program(nrows)
    in_maps = [{"inp": shards[i], **consts} for i in range(N_CORES)]
    res = run_bass_kernel_spmd(
        nc, in_maps, list(range(N_CORES)), trace=TRACE, tmpdir=TRACE_DIR
    )
    global LAST_RESULT
    LAST_RESULT = res

    out = np.empty((B, D), np.float32)
    for i in range(N_CORES):
        shard_t = np.asarray(res.results[i]["outT"])
        out[i * nrows : (i + 1) * nrows] = _unscramble(shard_t, nrows)
    return out


if __name__ == "__main__":
    rng = np.random.default_rng(0)
    B = N_CORES * DMA_ROWS * 2
    inp = rng.standard_normal((B, D)).astype(np.float32)
    w = rng.random((D, 3)).astype(np.float32)
    W_lin = (rng.standard_normal((D, D)) / np.sqrt(D)).astype(np.float32)
    b_lin = (rng.standard_normal(D) * 0.01).astype(np.float32)
    dt = np.ones(1, np.float32)

    actual = kernel(inp=inp, dt=dt, w=w, W_lin=W_lin, b_lin=b_lin)

    a = inp.astype(np.float64)
    c1 = (w[:, 0] * w[:, 2]).astype(np.float64)
    c2 = (w[:, 1] * w[:, 2]).astype(np.float64)
    c1[1] = w[1, 0] * w[1, 0]
    c2[1] = w[1, 1] * w[1, 0]
    ap1 = np.roll(a, -1, 1)
    am2 = np.roll(a, 2, 1)
    am1 = np.roll(a, 1, 1)
    x = (c1 * ap1 - c2 * am2) * am1
    expected = x + a @ W_lin.astype(np.float64).T + b_lin
    err = np.abs(actual - expected).max() / np.abs(expected).max()
    print("scale-relative absmax err:", err)
